# revision 13
# baseline (speedup 1.0000x reference)
"""Causal self-attention Trainium2 kernel (8 NeuronCores), v5.

Sharding: data-parallel over batch (2) x tensor-parallel over head groups
(12 heads -> 4 groups of 3). Core c handles batch c//4, head group c%4.
Each core computes its partial projection output (bf16); the host sums
the 4 partials per batch (TP reduce folded into the output gather).

All-bf16 compute.  v5 over v4:
  - input DMAs split across the two HWDGE queues (sync + scalar) and
    issued in 512-col pieces so the first QK fires ~3us in.
  - ACT exp table preloaded with a dummy activation during DMA wait.
  - rows 0-1 chunked at 512 so the pipeline starts after 2 qkv groups.
  - softmax normalize: den copy -> PE bcast into rows 64:128 of the SAME
    yq PSUM tile -> DVE recip -> DVE mul.  One PSUM buffer per AV, so
    the 4-buffer y/proj pool rotates freely; norm-finish is emitted as a
    deferred filler so the PE bcast never chases its own DVE den copy.
  - AV(3) split into 256-col halves: the low half only needs key rows
    <=13, so it completes (plus proj tiles 12-13) before the last two
    rows; only the high half's tail remains after the final exp.
"""

import functools

import numpy as np
import ml_dtypes

import concourse.bass as bass
import concourse.mybir as mybir
import concourse.tile as tile
from concourse import bacc
from concourse.bass_utils import run_bass_kernel_spmd
from concourse.masks import make_upper_triangular

P = 128
B, T, C = 2, 2048, 768
NH, HD = 12, 64
HPG = 3              # heads per core
NT = T // P          # 16 key tiles
NQ = T // 512        # 4 query chunks
QKW = 2 * HPG * HD   # 384 qk channels per core
VW = HPG * HD        # 192 v channels per core
F32 = mybir.dt.float32
F32R = mybir.dt.float32r
BF16 = mybir.dt.bfloat16
BF16NP = ml_dtypes.bfloat16

# exp row-block layout: row j at R3OFF[j], sub-blocks [h0|h1|h2], each
# W(j) = T-128j wide.
W = [T - P * j for j in range(NT)]
R3OFF = []
_o = 0
for _j in range(NT):
    R3OFF.append(_o)
    _o += HPG * W[_j]
EXPTOT = _o  # 52224

LAST_RESULT = None


def _emit(nc, tc, xT_d, wqk_d, wv_d, wp_d, out_d):
    from contextlib import ExitStack

    ctx = ExitStack()
    with ctx:
        const = ctx.enter_context(tc.tile_pool(name="const", bufs=1))
        tri_b = const.tile([P, P], BF16)
        make_upper_triangular(nc, tri_b[:], val=1.0, diag=True)
        tri3 = const.tile([P, HPG * P], BF16)
        for h in range(HPG):
            nc.vector.tensor_copy(
                out=tri3[:, h * P : (h + 1) * P], in_=tri_b[:]
            )
        ones_f32 = const.tile([P, HD], F32)
        nc.any.memset(ones_f32[:], 1.0)
        ones64 = const.tile([P, HD], F32R)
        nc.vector.tensor_copy(out=ones64[:], in_=ones_f32[:])
        # preload the ACT exp table set while DMAs run
        warm = const.tile([P, 8], F32)
        nc.scalar.activation(
            warm[:], ones_f32[:, 0:8],
            mybir.ActivationFunctionType.Exp, scale=1.0,
        )

        # ---- weights + xT in SBUF (DMAs split over both HWDGE queues) --
        dmaq = [nc.sync, nc.scalar]

        w_pool = ctx.enter_context(tc.tile_pool(name="w", bufs=1))
        wqk_sb = []
        for cc in range(6):
            t = w_pool.tile([P, QKW], BF16, tag=f"wqk{cc}")
            dmaq[cc % 2].dma_start(t[:], wqk_d[cc * P : (cc + 1) * P, :])
            wqk_sb.append(t)
        x_pool = ctx.enter_context(tc.tile_pool(name="x", bufs=1))
        xT_sb = []
        for cc in range(6):
            t = x_pool.tile([P, T], BF16, tag=f"x{cc}", name=f"x{cc}")
            xT_sb.append(t)
        for g0 in (0, 512):
            for cc in range(6):
                dmaq[cc % 2].dma_start(
                    xT_sb[cc][:, g0 : g0 + 512],
                    xT_d[cc * P : (cc + 1) * P, g0 : g0 + 512],
                )
        wv_sb = []
        for cc in range(6):
            t = w_pool.tile([P, VW], BF16, tag=f"wv{cc}")
            dmaq[cc % 2].dma_start(t[:], wv_d[cc * P : (cc + 1) * P, :])
            wv_sb.append(t)
        for cc in range(6):
            dmaq[cc % 2].dma_start(
                xT_sb[cc][:, 1024:T], xT_d[cc * P : (cc + 1) * P, 1024:T]
            )
        wpA = w_pool.tile([P, C], BF16, tag="wpA")
        nc.sync.dma_start(wpA[:], wp_d[0:P, :])
        wpB = w_pool.tile([HD, C], BF16, tag="wpB")
        nc.scalar.dma_start(wpB[:], wp_d[P : P + HD, :])

        # ---- persistent SBUF tensors ----------------------------------
        big = ctx.enter_context(tc.tile_pool(name="big", bufs=1))
        qkT = [
            big.tile([P, T], BF16, tag=f"qkT{m}", name=f"qkT{m}")
            for m in range(3)
        ]
        k2b = big.tile([HD, T], BF16, tag="k2b")
        v_sb = big.tile([P, HPG * NT * (HD + 1)], BF16, tag="vsb")
        nc.any.memset(v_sb[:], 1.0)  # ones cols at 64 mod 65 survive
        exp_sb = big.tile([P, EXPTOT], BF16, tag="exp")
        yT_a = big.tile([P, T], BF16, tag="ya")   # h0 rows 0:64, h1 64:128
        yT_b = big.tile([HD, T], BF16, tag="yb")  # h2

        nrm_pool = ctx.enter_context(tc.tile_pool(name="nrm", bufs=2))
        out_pool = ctx.enter_context(tc.tile_pool(name="outp", bufs=3))

        def blk(j):
            # [p, head(3), W[j]]
            return exp_sb[:, R3OFF[j] : R3OFF[j] + HPG * W[j]].rearrange(
                "p (h n) -> p h n", h=HPG
            )

        q_sl = [(qkT[0], 0), (qkT[0], HD), (qkT[2], 0)]
        k_sl = [(qkT[1], 0), (qkT[1], HD), (k2b, 0)]

        def ydst_of(h):
            return yT_a[0:HD, :] if h == 0 else (
                yT_a[HD:P, :] if h == 1 else yT_b[0:HD, :]
            )

        # ---------------- emission helpers ----------------
        ps_att = ctx.enter_context(
            tc.tile_pool(name="ps_att", bufs=1, space="PSUM")
        )

        def emit_qkv_group(ps_b, m, g):
            ps = ps_b.tile([P, 512], F32, tag="ab", bufs=2)
            for cc in range(6):
                nc.tensor.matmul(
                    ps[:],
                    wqk_sb[cc][:, m * P : (m + 1) * P],
                    xT_sb[cc][:, g * 512 : (g + 1) * 512],
                    start=(cc == 0),
                    stop=(cc == 5),
                )
            nc.vector.tensor_copy(
                out=qkT[m][:, g * 512 : (g + 1) * 512], in_=ps[:]
            )
            if m == 2:
                # rebase k2 (partitions 64:128) to partition 0 for h2 lhsT
                nc.sync.dma_start(
                    k2b[:, g * 512 : (g + 1) * 512],
                    qkT[2][HD:P, g * 512 : (g + 1) * 512],
                )

        def emit_v_group(ps_b, tt):
            ps = ps_b.tile([P, 512], F32, tag="ab", bufs=2)
            for cc in range(6):
                nc.tensor.matmul(
                    ps[:, 0:VW],
                    xT_sb[cc][:, tt * P : (tt + 1) * P],
                    wv_sb[cc][:, :],
                    start=(cc == 0),
                    stop=(cc == 5),
                )
            dst = v_sb[:].rearrange(
                "p (h t d) -> p h t d", h=HPG, t=NT, d=HD + 1
            )[:, :, tt, 0:HD]
            src = ps[:, 0:VW].rearrange("p (h d) -> p h d", h=HPG)
            nc.vector.tensor_copy(out=dst, in_=src)

        def emit_qk_unit(j, h, off, cw):
            # one (row, head, chunk): 1-2 QK matmuls + one exp call
            qlo = P * j + off
            qt, qo = q_sl[h]
            kt, ko = k_sl[h]
            st = ps_att.tile([P, 1024], F32, tag="st", bufs=2)
            for s0 in range(0, cw, 512):
                sw = min(512, cw - s0)
                nc.tensor.matmul(
                    st[:, s0 : s0 + sw],
                    kt[ko : ko + HD, P * j : P * j + P],
                    qt[qo : qo + HD, qlo + s0 : qlo + s0 + sw],
                    start=True,
                    stop=True,
                )
            dst = blk(j)[:, h, off : off + cw]
            nc.scalar.activation(
                dst, st[:, 0:cw], mybir.ActivationFunctionType.Exp,
                scale=0.125,
            )

        def emit_mask(j):
            dg = blk(j)[:, :, 0:P]
            t3 = tri3[:].rearrange("p (h n) -> p h n", h=HPG)
            nc.gpsimd.tensor_mul(out=dg, in0=dg, in1=t3)

        def vtile(h, jj):
            a = (h * NT + jj) * (HD + 1)
            return v_sb[:, a : a + (HD + 1)]

        # AV machinery: emit_av_mms computes the accumulation + den copy;
        # emit_norm_fin does bcast (into rows 64:128 of the same PSUM
        # tile), reciprocal, and the final normalize-multiply.
        av_state = {}

        def emit_av_mms(ps_c, key, qlo, width, h, jmax):
            yq = ps_c.tile([P, 512], F32, tag="y", bufs=4)
            first = True
            for jj in range(jmax + 1):
                woff = qlo - P * jj
                lo = max(0, -woff)
                n = width - lo
                if n <= 0:
                    continue
                rhs = blk(jj)[:, h, woff + lo : woff + lo + n]
                nc.tensor.matmul(
                    yq[0 : HD + 1, lo : lo + n],
                    vtile(h, jj),
                    rhs,
                    start=first,
                    stop=(jj == jmax),
                )
                first = False
            den = nrm_pool.tile([P, 512], F32R, tag="den")
            nc.vector.tensor_copy(
                out=den[HD : HD + 1, 0:width], in_=yq[HD : HD + 1, 0:width]
            )
            av_state[key] = (yq, den, qlo, width, h)

        def emit_norm_fin(key):
            # NOTE: DVE/ACT/GpSimd ops are partition-lane-aligned -- an op
            # can never move data across partitions, so the reciprocal and
            # multiply below keep everything on partitions 0:64 and only
            # the PE broadcast matmul moves the denominator row around.
            yq, den, qlo, width, h = av_state.pop(key)
            bc = ps_c.tile([P, 512], F32, tag="y", bufs=4)
            nc.tensor.matmul(
                bc[0:HD, 0:width],
                ones64[HD : HD + 1, :],
                den[HD : HD + 1, 0:width],
                start=True,
                stop=True,
            )
            bcs = nrm_pool.tile([HD, 512], F32, tag="bcs")
            with nc.allow_low_precision(reason="softmax denom"):
                nc.vector.reciprocal_approx_fast(
                    bcs[:, 0:width], bc[0:HD, 0:width]
                )
            nc.vector.tensor_mul(
                out=ydst_of(h)[:, qlo : qlo + width],
                in0=yq[0:HD, 0:width],
                in1=bcs[:, 0:width],
            )

        def emit_proj(ps_c, tt, cast_engine="dve"):
            pja = ps_c.tile([P, 512], F32, tag="y", bufs=4)
            pjb = ps_c.tile([P, 512], F32, tag="y", bufs=4)
            ysl = slice(tt * P, (tt + 1) * P)
            nc.tensor.matmul(
                pja[:], yT_a[:, ysl], wpA[:, 0:512], start=True, stop=False
            )
            nc.tensor.matmul(
                pjb[:, 0:256], yT_a[:, ysl], wpA[:, 512:C],
                start=True, stop=False,
            )
            nc.tensor.matmul(
                pja[:], yT_b[:, ysl], wpB[:, 0:512], start=False, stop=True
            )
            nc.tensor.matmul(
                pjb[:, 0:256], yT_b[:, ysl], wpB[:, 512:C],
                start=False, stop=True,
            )
            ot = out_pool.tile([P, C], BF16, tag="o")
            if cast_engine == "act":
                nc.scalar.copy(out=ot[:, 0:512], in_=pja[:])
                nc.scalar.copy(out=ot[:, 512:C], in_=pjb[:, 0:256])
            else:
                nc.vector.tensor_copy(out=ot[:, 0:512], in_=pja[:])
                nc.vector.tensor_copy(out=ot[:, 512:C], in_=pjb[:, 0:256])
            nc.sync.dma_start(out_d[tt * P : (tt + 1) * P, :], ot[:])

        # ---------------- schedule ----------------
        ps_b = tc.alloc_tile_pool(name="ps_b", bufs=1, space="PSUM")
        ps_c = None  # opened once phase-A psum pool is released

        # prelude: just enough for row 0 h0/h1
        emit_qkv_group(ps_b, 0, 0)
        emit_qkv_group(ps_b, 1, 0)

        # filler queue: (cost_ns, need_tag, fn)
        fillers = []
        fillers.append((1340, "m2g0", lambda: emit_qkv_group(ps_b, 2, 0)))
        for g in (1, 2, 3):
            fillers.append(
                (1340, f"m0g{g}", lambda g=g: emit_qkv_group(ps_b, 0, g))
            )
            fillers.append(
                (1340, f"m2g{g}", lambda g=g: emit_qkv_group(ps_b, 2, g))
            )
            fillers.append(
                (1340, f"m1g{g}", lambda g=g: emit_qkv_group(ps_b, 1, g))
            )
        for tt in range(NT):
            fillers.append(
                (480, f"v{tt}", lambda tt=tt: emit_v_group(ps_b, tt))
            )
        fi = 0
        done_tags = set()

        def pop_filler():
            nonlocal fi
            cost, tag, fn = fillers[fi]
            fi += 1
            fn()
            done_tags.add(tag)
            return cost

        def need(tag):
            while tag not in done_tags and fi < len(fillers):
                pop_filler()

        def drain():
            while fi < len(fillers):
                pop_filler()

        for j in range(NT):
            csz = 512 if j < 2 else 1024
            offs = []
            o = 0
            while o < W[j]:
                offs.append((o, min(csz, W[j] - o)))
                o += csz
            for off, cw in offs:
                for h in range(HPG):
                    g_need = (P * j + off + cw - 1) // 512
                    mwant = (0, 1) if h < 2 else (2,)
                    for m in mwant:
                        for g in range(g_need + 1):
                            if m != 1 or g <= j // 4:
                                need(f"m{m}g{g}")
                    emit_qk_unit(j, h, off, cw)
                    if off == 0 and h == 2:
                        emit_mask(j)
                    slack = cw * 0.417 + 220
                    while slack > 0 and fi < len(fillers):
                        slack -= pop_filler()
            if j == 3:
                # all qkv/v groups must be done before first AV (PSUM)
                drain()
                ps_b.release()
                ps_c = tc.alloc_tile_pool(name="ps_c", bufs=1, space="PSUM")
            if j in (3, 7, 11):
                q = j // 4
                cost = (4 * q + 2) * 215
                for h in range(HPG):
                    fillers.append(
                        (cost, f"av{q}h{h}",
                         lambda q=q, h=h: emit_av_mms(
                             ps_c, (q, h), 512 * q, 512, h, 4 * q + 3))
                    )
                    if h >= 1:
                        fillers.append(
                            (300, f"nf{q}h{h - 1}",
                             lambda q=q, h=h: emit_norm_fin((q, h - 1)))
                        )
                fillers.append(
                    (300, f"nf{q}h2", lambda q=q: emit_norm_fin((q, 2)))
                )
            if j in (5, 9):
                q = (j - 5) // 4
                for t4 in range(4):
                    fillers.append(
                        (660, f"pj{4 * q + t4}",
                         lambda q=q, t4=t4: emit_proj(ps_c, 4 * q + t4))
                    )
            if j == 13:
                for t4 in range(4):
                    fillers.append(
                        (660, f"pj{8 + t4}",
                         lambda t4=t4: emit_proj(ps_c, 8 + t4))
                    )
                drain()
                # AV(3) low half [1536:1792): needs rows <= 13 only
                emit_av_mms(ps_c, "A0", 1536, 256, 0, 13)
                emit_av_mms(ps_c, "A1", 1536, 256, 1, 13)
                emit_norm_fin("A0")
                emit_av_mms(ps_c, "A2", 1536, 256, 2, 13)
                emit_norm_fin("A1")
                emit_norm_fin("A2")
                emit_proj(ps_c, 12, cast_engine="dve")
                emit_proj(ps_c, 13, cast_engine="act")
                # pre-accumulate AV(3) high half for h0/h1 over rows 0..13
                yqB = {}
                for h in (0, 1):
                    yq = ps_c.tile([P, 512], F32, tag="y", bufs=4)
                    for jj in range(14):
                        woff = 1792 - P * jj
                        nc.tensor.matmul(
                            yq[0 : HD + 1, 0:256],
                            vtile(h, jj),
                            blk(jj)[:, h, woff : woff + 256],
                            start=(jj == 0),
                            stop=False,
                        )
                    yqB[h] = yq

        # ---- tail: finish AV(3) high half [1792:2048), proj 14/15 ----
        def finish_B(yq, h):
            for jj, (lo, n) in ((14, (0, 256)), (15, (128, 128))):
                woff = 1792 - P * jj
                nc.tensor.matmul(
                    yq[0 : HD + 1, lo : lo + n],
                    vtile(h, jj),
                    blk(jj)[:, h, woff + lo : woff + lo + n],
                    start=False,
                    stop=(jj == 15),
                )
            den = nrm_pool.tile([P, 512], F32R, tag="den")
            nc.vector.tensor_copy(
                out=den[HD : HD + 1, 0:256], in_=yq[HD : HD + 1, 0:256]
            )
            av_state[("B", h)] = (yq, den, 1792, 256, h)
            emit_norm_fin(("B", h))

        finish_B(yqB[0], 0)
        finish_B(yqB[1], 1)
        emit_av_mms(ps_c, "B2", 1792, 256, 2, 15)
        emit_norm_fin("B2")
        emit_proj(ps_c, 14, cast_engine="dve")
        emit_proj(ps_c, 15, cast_engine="act")
        ps_c.release()


@functools.cache
def _build():
    nc = bacc.Bacc(
        "TRN2",
        target_bir_lowering=False,
        debug=False,
        enable_asserts=False,
        num_devices=8,
    )
    xT_d = nc.dram_tensor("xt", [C, T], BF16, kind="ExternalInput").ap()
    wqk_d = nc.dram_tensor("wqk", [C, QKW], BF16, kind="ExternalInput").ap()
    wv_d = nc.dram_tensor("wv", [C, VW], BF16, kind="ExternalInput").ap()
    wp_d = nc.dram_tensor("wp", [VW, C], BF16, kind="ExternalInput").ap()
    out_d = nc.dram_tensor("out", [T, C], BF16, kind="ExternalOutput").ap()
    with tile.TileContext(nc) as tc:
        _emit(nc, tc, xT_d, wqk_d, wv_d, wp_d, out_d)
    nc.compile()
    return nc


def _host_inputs(x, Wqkv, Wproj):
    in_maps = []
    for c in range(8):
        b, g = divmod(c, 4)
        hs = [3 * g, 3 * g + 1, 3 * g + 2]

        def qcol(h):
            return Wqkv[:, 64 * h : 64 * h + 64]

        def kcol(h):
            return Wqkv[:, C + 64 * h : C + 64 * h + 64]

        def vcol(h):
            return Wqkv[:, 2 * C + 64 * h : 2 * C + 64 * h + 64]

        wqk = np.concatenate(
            [
                qcol(hs[0]), qcol(hs[1]),
                kcol(hs[0]), kcol(hs[1]),
                qcol(hs[2]), kcol(hs[2]),
            ],
            axis=1,
        )
        wv = np.concatenate([vcol(hs[0]), vcol(hs[1]), vcol(hs[2])], axis=1)
        wp = Wproj[VW * g : VW * (g + 1), :]
        in_maps.append(
            {
                "xt": np.ascontiguousarray(x[b].T).astype(BF16NP),
                "wqk": np.ascontiguousarray(wqk).astype(BF16NP),
                "wv": np.ascontiguousarray(wv).astype(BF16NP),
                "wp": np.ascontiguousarray(wp).astype(BF16NP),
            }
        )
    return in_maps


def kernel(x, mask, Wqkv, Wproj):
    global LAST_RESULT
    x = np.asarray(x, dtype=np.float32)
    Wqkv = np.asarray(Wqkv, dtype=np.float32)
    Wproj = np.asarray(Wproj, dtype=np.float32)

    in_maps = _host_inputs(x, Wqkv, Wproj)
    nc = _build()
    res = run_bass_kernel_spmd(nc, in_maps, core_ids=list(range(8)))
    LAST_RESULT = res
    out = np.empty((B, T, C), dtype=np.float32)
    for b in range(B):
        acc = res.results[4 * b]["out"].astype(np.float32)
        for g in range(1, 4):
            acc = acc + res.results[4 * b + g]["out"].astype(np.float32)
        out[b] = acc
    return out


if __name__ == "__main__":
    rng = np.random.default_rng(0)
    x = rng.standard_normal((B, T, C), dtype=np.float32)
    wqkv = rng.standard_normal((C, 3 * C), dtype=np.float32) / np.sqrt(C)
    wproj = rng.standard_normal((C, C), dtype=np.float32) / np.sqrt(C)
    o = kernel(x, None, wqkv, wproj)
    print(o.shape, o.dtype)
